# revision 11
# baseline (speedup 1.0000x reference)
"""EfficientDet post-processing (decode + class-aware NMS) on 8 Trainium2 cores.

Strategy: pure data parallel, one image per NeuronCore.

Per-core algorithm (exact-equivalent restructuring of the serial greedy NMS):
  1. Stream cls scores [49152, 80], segmented reduce_max -> per-anchor score.
  2. Threshold; per-partition top-8 (max8/max_index) -> 1024 candidates.
  3. Exact global sort of candidates by (score desc, idx asc) via pairwise
     rank counting; counting-scatter top-384 records into a sorted buffer.
     Candidate boxes are decoded from gathered loc/anchor rows only.
  4. Pairwise suppression matrix M_T[j,i] = (j before i) & (IoU > 0.2) on
     class-offset boxes (384x384, built with vector ops).
  5. Fixed-round closure iteration keep <- valid & (M_T.T @ keep == 0) with
     tiny PE matvecs (replaces the reference's 100-iteration serial loop).
  6. Output assembly: prefix-rank via triangular matmuls, zero-area repeat
     rule (a kept zero-area box fills all remaining slots, matching the
     reference's re-selection behavior), one indirect gather, pad fill.
"""
import sys
from contextlib import ExitStack

import numpy as np

try:
    import concourse.bass as bass
except Exception:
    sys.path.insert(0, "/opt/trn_rl_repo")
    import concourse.bass as bass

import concourse.bacc as bacc
import concourse.mybir as mybir
import concourse.tile as tile
from concourse.bass_utils import run_bass_kernel_spmd
from concourse.masks import make_identity

P = 128
F = 384
APAD = P * F          # 49152 (padded anchor count)
A = 49104
C = 80
TOPK = 8
NCAND = P * TOPK      # 1024
NSORT = 384
NCHUNKI = 3           # NSORT / 128
MAXD = 100
NCORES = 8
ROUNDS = 6            # closure rounds (validated: converges in <= 2)
REC = 16              # record: 0=score 1..4=ox1,oy1,ox2,oy2 5=area
                      #         6..9=x1,y1,x2,y2 10=label 11=gidx
CH = 48               # anchors per partition per streaming chunk
NCHUNK = F // CH
BIG = float(2.0 ** 32)
BIGV = 60000.0        # "never matches" sentinel for slot compares
SCORE_TH = 0.2
DUMP = NSORT + 1      # scatter dump row; NSORT row holds the pad record

f32 = mybir.dt.float32
i32 = mybir.dt.int32
u32 = mybir.dt.uint32
AX = mybir.AxisListType
OP = mybir.AluOpType
ACTF = mybir.ActivationFunctionType


def build_program(img_size: float) -> bass.Bass:
    lim = float(img_size) - 1.0
    two_size = 2.0 * float(img_size)

    nc = bacc.Bacc()
    cls_in = nc.dram_tensor("cls_in", [APAD, C], f32, kind="ExternalInput")
    raw_in = nc.dram_tensor("raw_in", [APAD, REC], f32, kind="ExternalInput")
    ob_out = nc.dram_tensor("out_boxes", [MAXD, 4], f32, kind="ExternalOutput")
    os_out = nc.dram_tensor("out_scores", [MAXD, 1], f32, kind="ExternalOutput")
    ol_out = nc.dram_tensor("out_labels", [MAXD, 1], i32, kind="ExternalOutput")

    with tile.TileContext(nc) as tc:
        with ExitStack() as ctx:
            sp = ctx.enter_context(tc.tile_pool(name="sp", bufs=1))
            cp = ctx.enter_context(tc.tile_pool(name="cp", bufs=3))
            jp = ctx.enter_context(tc.tile_pool(name="jp", bufs=2))
            pp = ctx.enter_context(tc.tile_pool(name="pp", bufs=1, space="PSUM"))
            dp = ctx.enter_context(tc.tile_pool(name="dp", bufs=1, space="DRAM"))

            # ---------------- constants
            idn = sp.tile([P, P], f32)
            make_identity(nc, idn[:])

            ones1p = sp.tile([1, P], f32)
            nc.vector.memset(ones1p[:], 1.0)
            ones11 = sp.tile([1, 1], f32)
            nc.vector.memset(ones11[:], 1.0)

            iota_cand_i = sp.tile([P, TOPK * C], i32)
            nc.gpsimd.iota(iota_cand_i[:], pattern=[[0, TOPK], [1, C]],
                           channel_multiplier=0)
            iota_cand = sp.tile([P, TOPK * C], f32)
            nc.vector.tensor_copy(iota_cand[:], iota_cand_i[:])

            rev_iota_i = sp.tile([P, NSORT], i32)
            nc.gpsimd.iota(rev_iota_i[:], pattern=[[-1, NSORT]], base=NSORT,
                           channel_multiplier=0)
            rev_iota = sp.tile([P, NSORT], f32)
            nc.vector.tensor_copy(rev_iota[:], rev_iota_i[:])

            kiota_i = sp.tile([P, 1], i32)
            nc.gpsimd.iota(kiota_i[:], pattern=[[0, 1]], channel_multiplier=1)
            kiota = sp.tile([P, 1], f32)
            nc.vector.tensor_copy(kiota[:], kiota_i[:])

            p384_i = sp.tile([P, 1], i32)
            nc.gpsimd.iota(p384_i[:], pattern=[[0, 1]], channel_multiplier=F)
            p384 = sp.tile([P, 1], f32)
            nc.vector.tensor_copy(p384[:], p384_i[:])

            ones_pn = sp.tile([P, NSORT], f32)
            nc.vector.memset(ones_pn[:], 1.0)
            lt_masks = []
            for t in range(NCHUNKI):
                ltm = sp.tile([P, NSORT], f32, name=f"ltm{t}")
                # keep 1.0 where (128t + p) < i  (iota = i - p - 128t > 0)
                nc.gpsimd.affine_select(
                    out=ltm[:], in_=ones_pn[:], pattern=[[1, NSORT]],
                    compare_op=OP.is_gt, fill=0.0,
                    base=-128 * t, channel_multiplier=-1)
                lt_masks.append(ltm)

            # ---------------- DRAM scratch
            s_dram = dp.tile([NCAND], f32, space="DRAM")
            i_dram = dp.tile([NCAND], f32, space="DRAM")
            sortbuf = dp.tile([NSORT + 2, REC], f32, space="DRAM")

            # pad record -> sortbuf row NSORT: score 0, box (0,0,1,1), label -1
            padr = sp.tile([1, REC], f32)
            nc.vector.memset(padr[:], 0.0)
            nc.vector.memset(padr[:, 8:10], 1.0)   # x2, y2
            nc.vector.memset(padr[:, 10:11], -1.0)  # label
            nc.sync.dma_start(sortbuf[NSORT:NSORT + 1, :], padr[:])

            # ---------------- phase 1: stream cls, per-anchor max score
            s = sp.tile([P, F], f32)
            cls_v = cls_in[:].rearrange("(p f) c -> p f c", p=P)
            for ch in range(NCHUNK):
                t_ = cp.tile([P, CH * C], f32, tag="stream")
                nc.gpsimd.dma_start(t_[:], cls_v[:, ch * CH:(ch + 1) * CH, :])
                nc.vector.tensor_reduce(
                    s[:, ch * CH:(ch + 1) * CH],
                    t_[:].rearrange("p (a c) -> p a c", c=C),
                    axis=AX.X, op=OP.max)

            # threshold: sthr = where(s > TH, s, -1)
            mask = sp.tile([P, F], u32)
            nc.vector.tensor_scalar(mask[:], s[:], SCORE_TH, None, op0=OP.is_gt)
            sthr = sp.tile([P, F], f32)
            nc.vector.memset(sthr[:], -1.0)
            nc.vector.copy_predicated(sthr[:], mask[:], s[:])

            # ---------------- phase 2: per-partition top-8 + global idx + ranks
            v = sp.tile([P, TOPK], f32)
            fidx = sp.tile([P, TOPK], u32)
            nc.vector.max_with_indices(v[:], fidx[:], sthr[:])

            fidx_f = sp.tile([P, TOPK], f32)
            nc.vector.tensor_copy(fidx_f[:], fidx[:])
            gidx_f = sp.tile([P, TOPK], f32)
            nc.vector.tensor_scalar(gidx_f[:], fidx_f[:], p384[:, 0:1], None,
                                    op0=OP.add)
            gidx_i = sp.tile([P, TOPK], i32)
            nc.vector.tensor_copy(gidx_i[:], gidx_f[:])

            nc.sync.dma_start(s_dram[:].rearrange("(p k) -> p k", p=P), v[:])
            nc.sync.dma_start(i_dram[:].rearrange("(p k) -> p k", p=P), gidx_f[:])

            s_all = sp.tile([P, NCAND], f32)
            nc.sync.dma_start(s_all[:], s_dram[:].unsqueeze(0).to_broadcast([P, NCAND]))
            i_all = sp.tile([P, NCAND], f32)
            nc.sync.dma_start(i_all[:], i_dram[:].unsqueeze(0).to_broadcast([P, NCAND]))

            rank = sp.tile([P, TOPK], f32)
            for k in range(TOPK):
                junk = jp.tile([P, NCAND], f32, tag="junk")
                rgt = jp.tile([P, 1], f32, tag="rgt")
                nc.vector.tensor_scalar(
                    out=junk[:], in0=s_all[:], scalar1=v[:, k:k + 1],
                    scalar2=None, op0=OP.is_gt, op1=OP.add, accum_out=rgt[:])
                eqm = jp.tile([P, NCAND], f32, tag="eqm")
                nc.vector.tensor_scalar(
                    out=eqm[:], in0=s_all[:], scalar1=v[:, k:k + 1],
                    scalar2=None, op0=OP.is_equal)
                junk2 = jp.tile([P, NCAND], f32, tag="junk2")
                tlt = jp.tile([P, 1], f32, tag="tlt")
                nc.vector.scalar_tensor_tensor(
                    out=junk2[:], in0=i_all[:], scalar=gidx_f[:, k:k + 1],
                    in1=eqm[:], op0=OP.is_lt, op1=OP.logical_and,
                    accum_out=tlt[:])
                nc.vector.tensor_tensor(rank[:, k:k + 1], rgt[:], tlt[:],
                                        op=OP.add)

            # clip: rank' = min(rank, NSORT) + (rank >= NSORT)  -> NSORT+1 dump
            rmin = sp.tile([P, TOPK], f32)
            nc.vector.tensor_scalar(rmin[:], rank[:], float(NSORT), None, op0=OP.min)
            rge = sp.tile([P, TOPK], f32)
            nc.vector.tensor_scalar(rge[:], rank[:], float(NSORT), None, op0=OP.is_ge)
            rclip = sp.tile([P, TOPK], f32)
            nc.vector.tensor_tensor(rclip[:], rmin[:], rge[:], op=OP.add)
            rclip_i = sp.tile([P, TOPK], i32)
            nc.vector.tensor_copy(rclip_i[:], rclip[:])

            # ---------------- phase 3: gather candidate rows, decode, build rec
            raw = sp.tile([P, TOPK, REC], f32)
            for k in range(TOPK):
                nc.gpsimd.indirect_dma_start(
                    out=raw[:, k, :], out_offset=None, in_=raw_in[:],
                    in_offset=bass.IndirectOffsetOnAxis(ap=gidx_i[:, k:k + 1], axis=0))

            clsg = sp.tile([P, TOPK * C], f32)
            for k in range(TOPK):
                nc.gpsimd.indirect_dma_start(
                    out=clsg[:, k * C:(k + 1) * C], out_offset=None, in_=cls_in[:],
                    in_offset=bass.IndirectOffsetOnAxis(ap=gidx_i[:, k:k + 1], axis=0))

            # candidate labels: enc = min_c((v_b - cls) * BIG + c)
            dcand = sp.tile([P, TOPK * C], f32)
            nc.vector.tensor_tensor(
                dcand[:].rearrange("p (k c) -> p k c", c=C),
                v[:].unsqueeze(2).to_broadcast([P, TOPK, C]),
                clsg[:].rearrange("p (k c) -> p k c", c=C),
                op=OP.subtract)
            nc.vector.scalar_tensor_tensor(
                out=dcand[:], in0=dcand[:], scalar=BIG, in1=iota_cand[:],
                op0=OP.mult, op1=OP.add)
            labf = sp.tile([P, TOPK], f32)
            nc.vector.tensor_reduce(
                labf[:], dcand[:].rearrange("p (k c) -> p k c", c=C),
                axis=AX.X, op=OP.min)

            # decode candidates ([P, TOPK] strided views into rec fields)
            rec = sp.tile([P, TOPK, REC], f32)
            nc.vector.memset(rec[:], 0.0)

            def rf(fld):
                return rec[:, :, fld]

            loc0, loc1, loc2, loc3 = (raw[:, :, j] for j in range(4))
            an0, an1, an2, an3 = (raw[:, :, 4 + j] for j in range(4))

            ya2 = sp.tile([P, TOPK], f32)
            nc.vector.tensor_tensor(ya2[:], an0, an2, op=OP.add)
            xa2 = sp.tile([P, TOPK], f32)
            nc.vector.tensor_tensor(xa2[:], an1, an3, op=OP.add)
            ha = sp.tile([P, TOPK], f32)
            nc.vector.tensor_tensor(ha[:], an2, an0, op=OP.subtract)
            wa = sp.tile([P, TOPK], f32)
            nc.vector.tensor_tensor(wa[:], an3, an1, op=OP.subtract)

            eh = sp.tile([P, TOPK], f32)
            nc.scalar.activation(eh[:], loc2, ACTF.Exp)
            ew = sp.tile([P, TOPK], f32)
            nc.scalar.activation(ew[:], loc3, ACTF.Exp)
            hh = sp.tile([P, TOPK], f32)
            nc.vector.tensor_tensor(hh[:], eh[:], ha[:], op=OP.mult)
            ww = sp.tile([P, TOPK], f32)
            nc.vector.tensor_tensor(ww[:], ew[:], wa[:], op=OP.mult)

            yc = sp.tile([P, TOPK], f32)
            nc.vector.tensor_tensor(yc[:], loc0, ha[:], op=OP.mult)
            nc.vector.scalar_tensor_tensor(
                out=yc[:], in0=ya2[:], scalar=0.5, in1=yc[:],
                op0=OP.mult, op1=OP.add)
            xc = sp.tile([P, TOPK], f32)
            nc.vector.tensor_tensor(xc[:], loc1, wa[:], op=OP.mult)
            nc.vector.scalar_tensor_tensor(
                out=xc[:], in0=xa2[:], scalar=0.5, in1=xc[:],
                op0=OP.mult, op1=OP.add)

            # x1,y1,x2,y2 clipped -> rec fields 6..9
            for fld, cc, sz, sign in ((6, xc, ww, -0.5), (7, yc, hh, -0.5),
                                      (8, xc, ww, 0.5), (9, yc, hh, 0.5)):
                tmp = jp.tile([P, TOPK], f32, tag="dectmp")
                nc.vector.scalar_tensor_tensor(
                    out=tmp[:], in0=sz[:], scalar=sign, in1=cc[:],
                    op0=OP.mult, op1=OP.add)
                nc.vector.tensor_scalar(rf(fld), tmp[:], 0.0, lim,
                                        op0=OP.max, op1=OP.min)

            # offset coords -> fields 1..4; area -> 5
            off = sp.tile([P, TOPK], f32)
            nc.vector.tensor_scalar(off[:], labf[:], two_size, None, op0=OP.mult)
            for fld in range(4):
                nc.vector.tensor_tensor(rf(1 + fld), rf(6 + fld), off[:], op=OP.add)
            aw_ = sp.tile([P, TOPK], f32)
            nc.vector.tensor_tensor(aw_[:], rf(3), rf(1), op=OP.subtract)
            ah_ = sp.tile([P, TOPK], f32)
            nc.vector.tensor_tensor(ah_[:], rf(4), rf(2), op=OP.subtract)
            nc.vector.tensor_tensor(rf(5), aw_[:], ah_[:], op=OP.mult)
            nc.vector.tensor_copy(rf(0), v[:])
            nc.vector.tensor_copy(rf(10), labf[:])
            nc.vector.tensor_copy(rf(11), gidx_f[:])

            # ---------------- phase 4: counting-scatter records to sortbuf
            for k in range(TOPK):
                nc.gpsimd.indirect_dma_start(
                    out=sortbuf[:], out_offset=bass.IndirectOffsetOnAxis(
                        ap=rclip_i[:, k:k + 1], axis=0),
                    in_=rec[:, k, :], in_offset=None)

            # ---------------- phase 5: suppression matrix M_T[j-part, i-free]
            rec_i = []      # sorted records, i-chunk layout [128, REC]
            for t in range(NCHUNKI):
                rt = sp.tile([P, REC], f32, name=f"rec_i{t}")
                nc.sync.dma_start(rt[:], sortbuf[128 * t:128 * (t + 1), :])
                rec_i.append(rt)

            # transpose records -> tf [REC, NSORT] (field rows)
            tf = sp.tile([REC, NSORT], f32)
            for t in range(NCHUNKI):
                tp_ = pp.tile([REC, P], f32, space="PSUM", tag="tfp")
                nc.tensor.transpose(tp_[:], in_=rec_i[t][:], identity=idn[:])
                nc.vector.tensor_copy(tf[:, 128 * t:128 * (t + 1)], tp_[:])

            # j-side broadcasts of ox1,oy1,ox2,oy2,area  (fields 1..5)
            jb = {}
            area_row = sp.tile([1, NSORT], f32)
            for fld in range(1, 6):
                # hop field row to partition 0 (matmul needs base partition 0)
                if fld == 5:
                    frow = area_row
                else:
                    frow = jp.tile([1, NSORT], f32, tag="frow")
                nc.sync.dma_start(frow[:], tf[fld:fld + 1, :])
                pb = pp.tile([P, NSORT], f32, space="PSUM", tag="jbp")
                nc.tensor.matmul(pb[:], lhsT=ones1p[:], rhs=frow[:],
                                 start=True, stop=True)
                sb_ = sp.tile([P, NSORT], f32, name=f"jb{fld}")
                nc.vector.tensor_copy(sb_[:], pb[:])
                jb[fld] = sb_

            mts = []
            for u in range(NCHUNKI):
                ru = rec_i[u]
                a1 = jp.tile([P, NSORT], f32, tag="a1")
                nc.vector.tensor_scalar(a1[:], jb[3][:], ru[:, 3:4], None, op0=OP.min)
                b1 = jp.tile([P, NSORT], f32, tag="b1")
                nc.vector.tensor_scalar(b1[:], jb[1][:], ru[:, 1:2], None, op0=OP.max)
                iw = jp.tile([P, NSORT], f32, tag="iw")
                nc.vector.tensor_tensor(iw[:], a1[:], b1[:], op=OP.subtract)
                nc.scalar.activation(iw[:], iw[:], ACTF.Relu)

                a2 = jp.tile([P, NSORT], f32, tag="a2")
                nc.vector.tensor_scalar(a2[:], jb[4][:], ru[:, 4:5], None, op0=OP.min)
                b2 = jp.tile([P, NSORT], f32, tag="b2")
                nc.vector.tensor_scalar(b2[:], jb[2][:], ru[:, 2:3], None, op0=OP.max)
                ih = jp.tile([P, NSORT], f32, tag="ih")
                nc.vector.tensor_tensor(ih[:], a2[:], b2[:], op=OP.subtract)
                nc.scalar.activation(ih[:], ih[:], ACTF.Relu)

                inter = jp.tile([P, NSORT], f32, tag="inter")
                nc.vector.tensor_tensor(inter[:], iw[:], ih[:], op=OP.mult)
                d1 = jp.tile([P, NSORT], f32, tag="d1")
                nc.vector.tensor_scalar(d1[:], jb[5][:], ru[:, 5:6], 1e-8,
                                        op0=OP.add, op1=OP.add)
                cmp_ = jp.tile([P, NSORT], f32, tag="cmp")
                nc.vector.scalar_tensor_tensor(
                    out=cmp_[:], in0=inter[:], scalar=6.0, in1=d1[:],
                    op0=OP.mult, op1=OP.is_gt)
                mt = sp.tile([P, NSORT], f32, name=f"mt{u}")
                nc.gpsimd.affine_select(
                    out=mt[:], in_=cmp_[:], pattern=[[1, NSORT]],
                    compare_op=OP.is_gt, fill=0.0,
                    base=-128 * u, channel_multiplier=-1)
                mts.append(mt)

            # ---------------- phase 6: closure rounds
            valid = []
            for t in range(NCHUNKI):
                vt = sp.tile([P, 1], f32, name=f"valid{t}")
                nc.vector.tensor_scalar(vt[:], rec_i[t][:, 0:1], SCORE_TH, None,
                                        op0=OP.is_gt)
                valid.append(vt)

            keep = []
            for t in range(NCHUNKI):
                kt = sp.tile([P, 1], f32, name=f"keep{t}")
                nc.vector.tensor_copy(kt[:], valid[t][:])
                keep.append(kt)

            for _ in range(ROUNDS):
                newk = []
                for t in range(NCHUNKI):
                    et = pp.tile([P, 1], f32, space="PSUM", tag="et")
                    for u in range(NCHUNKI):
                        nc.tensor.matmul(
                            et[:], lhsT=mts[u][:, 128 * t:128 * (t + 1)],
                            rhs=keep[u][:], start=(u == 0), stop=(u == NCHUNKI - 1))
                    nk = jp.tile([P, 1], f32, tag=f"nk{t}")
                    nc.vector.scalar_tensor_tensor(
                        out=nk[:], in0=et[:], scalar=0.5, in1=valid[t][:],
                        op0=OP.is_lt, op1=OP.logical_and)
                    newk.append(nk)
                for t in range(NCHUNKI):
                    nc.vector.tensor_copy(keep[t][:], newk[t][:])

            # ---------------- phase 7: output assembly
            # keep row [1, NSORT]
            krow = sp.tile([1, NSORT], f32)
            for t in range(NCHUNKI):
                tpk = pp.tile([1, P], f32, space="PSUM", tag="tpk")
                nc.tensor.transpose(tpk[:], in_=keep[t][:], identity=idn[:])
                nc.vector.tensor_copy(krow[:, 128 * t:128 * (t + 1)], tpk[:])

            # z / zseen / keep_eff / zfirst rows
            zrow = sp.tile([1, NSORT], f32)
            nc.vector.scalar_tensor_tensor(
                out=zrow[:], in0=area_row[:], scalar=0.0, in1=krow[:],
                op0=OP.is_equal, op1=OP.logical_and)
            zs = sp.tile([1, NSORT], f32)
            nc.vector.tensor_tensor_scan(
                out=zs[:], data0=zrow[:], data1=zrow[:], initial=0.0,
                op0=OP.max, op1=OP.max)
            ze = sp.tile([1, NSORT], f32)
            nc.vector.memset(ze[:], 0.0)
            nc.vector.tensor_copy(ze[:, 1:NSORT], zs[:, 0:NSORT - 1])
            nzex = sp.tile([1, NSORT], f32)
            nc.vector.tensor_scalar(nzex[:], ze[:], 0.5, None, op0=OP.is_lt)
            keff_row = sp.tile([1, NSORT], f32)
            nc.vector.tensor_tensor(keff_row[:], krow[:], nzex[:], op=OP.logical_and)
            zfirst_row = sp.tile([1, NSORT], f32)
            nc.vector.tensor_tensor(zfirst_row[:], zrow[:], nzex[:], op=OP.logical_and)

            # keep_eff cols
            keff_col = []
            for t in range(NCHUNKI):
                pc = pp.tile([P, 1], f32, space="PSUM", tag="pc")
                nc.tensor.matmul(pc[:], lhsT=keff_row[:, 128 * t:128 * (t + 1)],
                                 rhs=ones11[:], start=True, stop=True)
                kc = sp.tile([P, 1], f32, name=f"keffc{t}")
                nc.vector.tensor_copy(kc[:], pc[:])
                keff_col.append(kc)

            # prefix ranks (exclusive) over keep_eff
            ppr = pp.tile([1, NSORT], f32, space="PSUM", tag="ppr")
            for t in range(NCHUNKI):
                nc.tensor.matmul(ppr[:], lhsT=keff_col[t][:], rhs=lt_masks[t][:],
                                 start=(t == 0), stop=(t == NCHUNKI - 1))
            pr = sp.tile([1, NSORT], f32)
            nc.vector.tensor_copy(pr[:], ppr[:])

            keffu = sp.tile([1, NSORT], u32)
            nc.vector.tensor_copy(keffu[:], keff_row[:])
            zfirstu = sp.tile([1, NSORT], u32)
            nc.vector.tensor_copy(zfirstu[:], zfirst_row[:])
            prq = sp.tile([1, NSORT], f32)
            nc.vector.memset(prq[:], BIGV)
            nc.vector.copy_predicated(prq[:], keffu[:], pr[:])
            zrq = sp.tile([1, NSORT], f32)
            nc.vector.memset(zrq[:], BIGV)
            nc.vector.copy_predicated(zrq[:], zfirstu[:], pr[:])

            # broadcast both rows, build one-hot, fold to gather offsets
            prb = pp.tile([P, NSORT], f32, space="PSUM", tag="prb")
            nc.tensor.matmul(prb[:], lhsT=ones1p[:], rhs=prq[:], start=True, stop=True)
            zrb = pp.tile([P, NSORT], f32, space="PSUM", tag="zrb")
            nc.tensor.matmul(zrb[:], lhsT=ones1p[:], rhs=zrq[:], start=True, stop=True)

            oh = jp.tile([P, NSORT], f32, tag="oh")
            nc.vector.tensor_scalar(oh[:], prb[:], kiota[:, 0:1], None, op0=OP.is_equal)
            o2 = jp.tile([P, NSORT], f32, tag="o2")
            nc.vector.tensor_scalar(o2[:], zrb[:], kiota[:, 0:1], None, op0=OP.is_le)
            ohh = jp.tile([P, NSORT], f32, tag="ohh")
            nc.vector.tensor_tensor(ohh[:], oh[:], o2[:], op=OP.logical_or)

            junk3 = jp.tile([P, NSORT], f32, tag="junk3")
            gs = sp.tile([P, 1], f32)
            nc.vector.scalar_tensor_tensor(
                out=junk3[:], in0=ohh[:], scalar=1.0, in1=rev_iota[:],
                op0=OP.bypass, op1=OP.mult, accum_out=gs[:])
            gcol = sp.tile([P, 1], f32)
            nc.vector.tensor_scalar(gcol[:], gs[:], -1.0, float(NSORT),
                                    op0=OP.mult, op1=OP.add)
            gcol_i = sp.tile([P, 1], i32)
            nc.vector.tensor_copy(gcol_i[:], gcol[:])

            frec = sp.tile([P, REC], f32)
            nc.gpsimd.indirect_dma_start(
                out=frec[:], out_offset=None, in_=sortbuf[:],
                in_offset=bass.IndirectOffsetOnAxis(ap=gcol_i[:, 0:1], axis=0))

            lab_i = sp.tile([P, 1], i32)
            nc.vector.tensor_copy(lab_i[:], frec[:, 10:11])

            nc.sync.dma_start(ob_out[:], frec[0:MAXD, 6:10])
            nc.sync.dma_start(os_out[:], frec[0:MAXD, 0:1])
            nc.sync.dma_start(ol_out[:], lab_i[0:MAXD, :])

    nc.finalize()
    return nc


_CACHE = {}


def _get_program(img_size: float) -> bass.Bass:
    key = float(img_size)
    if key not in _CACHE:
        _CACHE[key] = build_program(key)
    return _CACHE[key]


def prep_inputs(cls_preds, loc_preds, anchors):
    """Host-side shard prep: pad + concat layout only (no computation)."""
    cls_preds = np.asarray(cls_preds, dtype=np.float32)
    loc_preds = np.asarray(loc_preds, dtype=np.float32)
    anchors = np.asarray(anchors, dtype=np.float32)
    B, A_, C_ = cls_preds.shape
    in_maps = []
    for b in range(B):
        cls_p = np.zeros((APAD, C), np.float32)
        cls_p[:A_] = cls_preds[b]
        raw = np.zeros((APAD, REC), np.float32)
        raw[:A_, 0:4] = loc_preds[b]
        raw[:A_, 4:8] = anchors
        in_maps.append({"cls_in": cls_p, "raw_in": raw})
    return in_maps


def kernel(cls_preds, loc_preds, anchors, img_size):
    nc = _get_program(float(img_size))
    in_maps = prep_inputs(cls_preds, loc_preds, anchors)
    B = len(in_maps)
    res = run_bass_kernel_spmd(nc, in_maps, core_ids=list(range(B))).results
    out_boxes = np.stack([r["out_boxes"] for r in res])           # [B,100,4]
    out_scores = np.stack([r["out_scores"][:, 0] for r in res])   # [B,100]
    out_labels = np.stack([r["out_labels"][:, 0] for r in res])   # [B,100] i32
    return out_boxes, out_scores, out_labels.astype(np.int32)
